# revision 11
# baseline (speedup 1.0000x reference)
"""GCN feature extractor (GCNConv + BatchNorm1d + ReLU) as a Trainium2 Bass kernel.

Distribution (8 NeuronCores):
  - Nodes are sharded row-wise across the 8 cores (graph/data parallel).
  - Each core computes m = deg^-1/2 * (x @ W) for its own node shard (PE
    matmul, bf16).  The shard is split into two halves (A: 25 tiles, B: 24
    tiles) which are AllGather'd separately so that gathers/aggregation of
    the A window overlap the B AllGather (collectives run on the CC cores,
    leaving all compute engines free).
  - Gather-table blocks are stored p-major ([part, tile, feat]) so the
    SBUF->DRAM repack is a single full-rate DMA; the host index formula
    absorbs the layout.
  - Each core owns the edges whose TARGET falls in its shard.  Per 128-target
    tile it bulk-gathers the source messages with the GpSimd descriptor-
    generated gather DMA (dma_gather), builds one-hot target-selection
    matrices on the vector engine, and reduces on the tensor engine:
        agg[f, t] += G[edge, f]^T @ onehot[edge, t]   (PSUM fp32 accumulate)
    Descriptor generation is round-robined over 4 SWDGE queues (4 Q7 core
    pairs generate concurrently).  Aggregation is two-pass (pass 1: self-loop
    + window A into SBUF; pass 2: + window B) so pass 1 runs during the B
    AllGather.
  - Self loops are folded in locally (m_local tile x identity matmul); the
    symmetric normalization factorizes as deg^-1/2[src] (folded into m) and
    deg^-1/2[tgt] (applied at the end).  +bias cancels under BatchNorm.
  - BatchNorm statistics are computed per-feature batched at the end,
    AllReduce'd across cores, applied fused with ReLU in one scalar-engine
    activation; output is written feature-major and transposed on the host.
"""

import sys

sys.path.insert(0, "/opt/trn_rl_repo")

import numpy as np
import ml_dtypes

import os
import concourse.bass as bass
import concourse.tile as tile
from concourse import bacc, mybir, library_config
from concourse.bass_utils import run_bass_kernel_spmd

N_CORES = 8
P = 128
GK = 8   # gather-DMA granularity: blocks (of 128 edges) per dma_gather call
NQ = 4   # SWDGE queues (Q7 core pairs) for gather descriptor generation
BN_EPS = 1e-5
dt = mybir.dt


# ---------------------------------------------------------------- host prep
def _plan_and_pack(x, edge_index, W, gamma, beta):
    N, IN = x.shape
    HID = W.shape[1]
    assert HID == P and IN % P == 0
    shard = (N + N_CORES - 1) // N_CORES          # nodes per core (last may be short)
    PS = ((shard + P - 1) // P) * P               # padded shard rows
    NT = PS // P                                  # 128-target tiles per core
    NTA = (NT + 1) // 2                           # tiles in window A
    NTB = NT - NTA
    RA = NTA * P                                  # rows in window A per shard
    sizA, sizB = N_CORES * RA, N_CORES * (PS - RA)
    assert sizA < 2 ** 15 and sizB < 2 ** 15, "int16 gather index overflow"

    row = np.asarray(edge_index[0], dtype=np.int64)
    col = np.asarray(edge_index[1], dtype=np.int64)

    deg = np.bincount(col, minlength=N).astype(np.float64) + 1.0
    dis = (1.0 / np.sqrt(deg)).astype(np.float32)

    # window + p-major gather-table coordinate of each node
    # table block c is [128 part, T tiles, 128 feat]; node (c, r) with
    # r = t*128 + p sits at flat row c*T*128 + p*T + t of the window table.
    cc = np.arange(N) // shard
    rr = np.arange(N) % shard
    win = (rr >= RA).astype(np.int64)
    rA = rr
    tblA = cc * RA + (rA % P) * NTA + rA // P
    rB = rr - RA
    tblB = cc * (PS - RA) + (rB % P) * NTB + rB // P
    tbl = np.where(win == 0, tblA, tblB).astype(np.int32)

    # self loops are folded locally on-device; only real edges get gathered
    order = np.argsort(col, kind="stable")
    allr = row[order]
    allc = col[order]
    src_tbl_all = tbl[allr]
    win_of = win[allr]

    tile_of = allc // shard * NT + (allc % shard) // P
    tloc_of = (allc % shard) % P

    n_tiles_total = N_CORES * NT
    key = tile_of * 2 + win_of
    sort2 = np.argsort(key, kind="stable")
    src_sorted = src_tbl_all[sort2]
    tloc_sorted = tloc_of[sort2].astype(np.int32)
    key_sorted = key[sort2]
    bounds = np.searchsorted(key_sorted, np.arange(n_tiles_total * 2 + 1))

    # shared (max over cores) block counts per (tile, window)
    nb = np.zeros((N_CORES, NT, 2), np.int64)
    for c in range(N_CORES):
        for t in range(NT):
            for w in range(2):
                k = (c * NT + t) * 2 + w
                cnt = bounds[k + 1] - bounds[k]
                nb[c, t, w] = (cnt + P - 1) // P
    nbmax = nb.max(axis=0)                         # [NT, 2]
    # stream block offsets: stream w blocks of tile t start at soff[t, w]
    soff = np.zeros((NT, 2), np.int64)
    tb = [0, 0]
    for t in range(NT):
        for w in range(2):
            soff[t, w] = tb[w]
            tb[w] += nbmax[t, w]
    TB0, TB1 = int(tb[0]), int(tb[1])
    ncalls = [(TB0 + GK - 1) // GK, (TB1 + GK - 1) // GK]
    TBpad = [ncalls[0] * GK, ncalls[1] * GK]

    per_core = []
    for c in range(N_CORES):
        streams = [np.zeros(TBpad[w] * P, np.int32) for w in range(2)]
        ctgtl = -np.ones((P, TB0 + TB1), np.float32)
        for t in range(NT):
            co = int(soff[t, 0] + soff[t, 1])
            done = 0
            for w in range(2):
                k = (c * NT + t) * 2 + w
                lo, hi = bounds[k], bounds[k + 1]
                b0 = soff[t, w]
                streams[w][b0 * P: b0 * P + (hi - lo)] = src_sorted[lo:hi]
                n = hi - lo
                if n:
                    jj = np.arange(n)
                    ctgtl[jj % P, co + done + jj // P] = tloc_sorted[lo:hi]
                done += int(nbmax[t, w])
        # pack gather indices: per call [128, GK*128/16] int16, idx j -> [16c + j%16, j//16]
        idxs = []
        for w in range(2):
            s16 = streams[w].astype(np.int16)
            a = s16.reshape(ncalls[w], GK * P // 16, 16).transpose(0, 2, 1)  # [calls, 16, cols]
            a = np.tile(a, (1, 8, 1))                                        # replicate to 128 partitions
            idxs.append(np.ascontiguousarray(a.transpose(1, 0, 2).reshape(P, -1)))

        lo_n = c * shard
        hi_n = min((c + 1) * shard, N)
        ns = hi_n - lo_n
        xs = np.zeros((IN, PS), np.float32)
        xs[:, :ns] = x[lo_n:hi_n].T
        dis_s = np.zeros(PS, np.float32)
        dis_s[:ns] = dis[lo_n:hi_n]
        per_core.append({
            "xT": xs.astype(ml_dtypes.bfloat16),
            "disb": np.ascontiguousarray(np.tile(dis_s[None, :], (P, 1))),
            "disk": np.ascontiguousarray(dis_s.reshape(NT, P).T),   # [128, NT]
            "idx0": idxs[0], "idx1": idxs[1],
            "ctgtl": ctgtl.astype(ml_dtypes.bfloat16),
            "W": np.ascontiguousarray(W.astype(ml_dtypes.bfloat16)),
            "iota": np.ascontiguousarray(
                np.tile(np.arange(P, dtype=np.float32), (P, 1)).astype(ml_dtypes.bfloat16)),
            "eye": np.eye(P, dtype=ml_dtypes.bfloat16),
            "gamma": np.ascontiguousarray(gamma.astype(np.float32).reshape(P, 1)),
            "beta": np.ascontiguousarray(beta.astype(np.float32).reshape(P, 1)),
        })

    plan = {
        "N": N, "IN": IN, "PS": PS, "NT": NT, "NTA": NTA, "shard": shard,
        "sizA": sizA, "sizB": sizB,
        "nbmax": nbmax, "soff": soff, "TB": [TB0, TB1], "ncalls": ncalls,
        "KC": IN // P,
    }
    return plan, per_core


# ---------------------------------------------------------------- bass build
def _build(plan):
    N, IN, PS, NT = plan["N"], plan["IN"], plan["PS"], plan["NT"]
    KC = plan["KC"]
    NTA = plan["NTA"]
    NTB = NT - NTA
    sizA, sizB = plan["sizA"], plan["sizB"]
    nbmax, soff = plan["nbmax"], plan["soff"]
    ncalls = plan["ncalls"]
    NIDX = GK * P

    nc = bacc.Bacc("TRN2", target_bir_lowering=False, debug=False,
                   num_devices=N_CORES, num_swdge_queues=NQ)
    t_xT = nc.dram_tensor("xT", [IN, PS], dt.bfloat16, kind="ExternalInput").ap()
    t_W = nc.dram_tensor("W", [IN, P], dt.bfloat16, kind="ExternalInput").ap()
    t_disb = nc.dram_tensor("disb", [P, PS], dt.float32, kind="ExternalInput").ap()
    t_disk = nc.dram_tensor("disk", [P, NT], dt.float32, kind="ExternalInput").ap()
    t_idx = [nc.dram_tensor(f"idx{w}", [P, ncalls[w] * NIDX // 16], dt.int16,
                            kind="ExternalInput").ap() for w in range(2)]
    t_ctgtl = nc.dram_tensor("ctgtl", [P, plan["TB"][0] + plan["TB"][1]],
                             dt.bfloat16, kind="ExternalInput").ap()
    t_iota = nc.dram_tensor("iota", [P, P], dt.bfloat16, kind="ExternalInput").ap()
    t_eye = nc.dram_tensor("eye", [P, P], dt.bfloat16, kind="ExternalInput").ap()
    t_gamma = nc.dram_tensor("gamma", [P, 1], dt.float32, kind="ExternalInput").ap()
    t_beta = nc.dram_tensor("beta", [P, 1], dt.float32, kind="ExternalInput").ap()
    t_out = nc.dram_tensor("out_t", [P, PS], dt.float32, kind="ExternalOutput").ap()

    INV_N = 1.0 / N

    with tile.TileContext(nc) as tc:
        nc.gpsimd.load_library(library_config.mlp)
        with tc.tile_pool(name="consts", bufs=1) as cst, \
             tc.tile_pool(name="gp0", bufs=14) as gp0, \
             tc.tile_pool(name="gp1", bufs=14) as gp1, \
             tc.tile_pool(name="ohp", bufs=3) as ohp, \
             tc.tile_pool(name="big", bufs=1) as big, \
             tc.tile_pool(name="ep", bufs=3) as ep, \
             tc.tile_pool(name="hps", bufs=1, space="PSUM") as hps, \
             tc.tile_pool(name="aps", bufs=6, space="PSUM") as aps, \
             tc.tile_pool(name="stp", bufs=1) as stp, \
             tc.tile_pool(name="dram", bufs=1, space="DRAM") as dram:

            # warmup: tiny first collective so the one-time cross-core
            # rendezvous overlaps the input loads and phase B
            wu_in = dram.tile([P, 1], dt.float32)
            wu_out = dram.tile([P, 1], dt.float32, addr_space="Shared")
            nc.gpsimd.collective_compute(
                "AllReduce", mybir.AluOpType.add,
                replica_groups=[list(range(N_CORES))],
                ins=[wu_in[:]], outs=[wu_out[:]])

            # ---- constants to SBUF
            W_sb = cst.tile([P, KC, P], dt.bfloat16)
            for k in range(KC):
                nc.sync.dma_start(out=W_sb[:, k, :], in_=t_W[k * P:(k + 1) * P, :])
            disk_sb = cst.tile([P, NT], dt.float32)
            nc.sync.dma_start(out=disk_sb[:], in_=t_disk[:])
            # bulk xT load: 2 big DMAs instead of 98 tile loads
            # (allocated from the rotating "big" pool so the square buffer
            #  can reuse the same bytes once phase B has consumed xT)
            xT_sb = big.tile([P, KC, PS], dt.bfloat16, name="big")
            for q in range(KC):
                nc.sync.dma_start(out=xT_sb[:, q, :],
                                  in_=t_xT[q * P:(q + 1) * P, :])

            # ---- phase B: m = dis * (x @ W), bf16, kept in SBUF (p-major)
            m_all = cst.tile([P, NT, P], dt.bfloat16)

            def phase_b(t0, t1):
                for k in range(t0, t1):
                    h_ps = hps.tile([P, P], dt.float32, name="hps")
                    for q in range(KC):
                        nc.tensor.matmul(out=h_ps[:],
                                         lhsT=xT_sb[:, q, k * P:(k + 1) * P],
                                         rhs=W_sb[:, q, :],
                                         start=(q == 0), stop=(q == KC - 1))
                    nc.vector.tensor_scalar(out=m_all[:, k, :], in0=h_ps[:],
                                            scalar1=disk_sb[:, k:k + 1],
                                            scalar2=None,
                                            op0=mybir.AluOpType.mult)

            # window A: compute, repack (one DMA, p-major), AllGather
            m_ccA = dram.tile([P, NTA * P], dt.bfloat16)
            m_fullA = dram.tile([sizA, P], dt.bfloat16, addr_space="Shared")
            phase_b(0, NTA)
            nc.sync.dma_start(out=m_ccA[:], in_=m_all[:, 0:NTA, :])
            nc.gpsimd.collective_compute(
                "AllGather", mybir.AluOpType.bypass,
                replica_groups=[list(range(N_CORES))],
                ins=[m_ccA[:]], outs=[m_fullA[:]])

            # gather/agg metadata loads issue only now, so they don't delay
            # the phase-B inputs and the first AllGather on the DMA queues
            iota_sb = cst.tile([P, P], dt.bfloat16)
            nc.sync.dma_start(out=iota_sb[:], in_=t_iota[:])
            eye_sb = cst.tile([P, P], dt.bfloat16)
            nc.sync.dma_start(out=eye_sb[:], in_=t_eye[:])
            idx_sb = [cst.tile([P, ncalls[w] * NIDX // 16], dt.int16, name=f"idx{w}")
                      for w in range(2)]
            for w in range(2):
                nc.sync.dma_start(out=idx_sb[w][:], in_=t_idx[w][:])
            TBC = plan["TB"][0] + plan["TB"][1]
            ctgtl_sb = cst.tile([P, TBC], dt.bfloat16)
            nc.sync.dma_start(out=ctgtl_sb[:], in_=t_ctgtl[:])
            disb_sb = cst.tile([P, PS], dt.float32)
            nc.sync.dma_start(out=disb_sb[:], in_=t_disb[:])
            gamma_sb = cst.tile([P, 1], dt.float32)
            nc.sync.dma_start(out=gamma_sb[:], in_=t_gamma[:])
            beta_sb = cst.tile([P, 1], dt.float32)
            nc.sync.dma_start(out=beta_sb[:], in_=t_beta[:])

            # window B
            m_ccB = dram.tile([P, NTB * P], dt.bfloat16)
            m_fullB = dram.tile([sizB, P], dt.bfloat16, addr_space="Shared")
            phase_b(NTA, NT)
            nc.sync.dma_start(out=m_ccB[:], in_=m_all[:, NTA:NT, :])
            nc.gpsimd.collective_compute(
                "AllGather", mybir.AluOpType.bypass,
                replica_groups=[list(range(N_CORES))],
                ins=[m_ccB[:]], outs=[m_fullB[:]])

            # ---- gather pipelines (two windows, round-robin SWDGE queues)
            g_tiles = [[], []]
            gpools = [gp0, gp1]
            m_wins = [m_fullA, m_fullB]
            qctr = 0

            def issue_gathers(w):
                nonlocal qctr
                for cidx in range(ncalls[w]):
                    gt = gpools[w].tile([P, GK, P], dt.bfloat16, name=f"g{w}")
                    nc.gpsimd.dma_gather(
                        out_ap=gt[:],
                        in_ap=m_wins[w][:],
                        idxs_ap=idx_sb[w][:, cidx * NIDX // 16:(cidx + 1) * NIDX // 16],
                        num_idxs=NIDX, num_idxs_reg=NIDX, elem_size=P,
                        queue_num=qctr % NQ)
                    qctr += 1
                    g_tiles[w].append(gt)

            opre_all = stp.tile([P, NT * P], dt.float32)
            OHMX = int(nbmax.sum(axis=1).max())

            issue_gathers(0)
            issue_gathers(1)

            for t in range(NT):
                nb0, nb1 = int(nbmax[t, 0]), int(nbmax[t, 1])
                nbt = nb0 + nb1
                co = int(soff[t, 0] + soff[t, 1])
                ps_t = aps.tile([P, P], dt.float32, name="agg")
                # self-loop fold: out[f, t] += m_local[t, f]
                nc.tensor.matmul(out=ps_t[:], lhsT=m_all[:, t, :],
                                 rhs=eye_sb[:], start=True, stop=(nbt == 0))
                if nbt:
                    oh = ohp.tile([P, OHMX, P], dt.bfloat16, name="oh")
                    nc.vector.tensor_tensor(
                        out=oh[:, 0:nbt, :],
                        in0=ctgtl_sb[:, co:co + nbt].unsqueeze(2)
                            .to_broadcast([P, nbt, P]),
                        in1=iota_sb[:].unsqueeze(1).to_broadcast([P, nbt, P]),
                        op=mybir.AluOpType.is_equal)
                    for b in range(nbt):
                        w = 0 if b < nb0 else 1
                        j = int(soff[t, w]) + (b if w == 0 else b - nb0)
                        gt = g_tiles[w][j // GK]
                        nc.tensor.matmul(
                            out=ps_t[:], lhsT=gt[:, j % GK, :],
                            rhs=oh[:, b, :],
                            start=False, stop=(b == nbt - 1))
                nc.vector.tensor_copy(out=opre_all[:, t * P:(t + 1) * P],
                                      in_=ps_t[:])

            # ---- scale by dis[tgt], batched stats
            st_sb = stp.tile([P, 2], dt.float32)
            s1p = stp.tile([P, 2], dt.float32)
            s2p = stp.tile([P, 2], dt.float32)
            sq_all = big.tile([P, NT * P], dt.float32, name="big")
            HNT = NT // 2
            for ci, (a, b_) in enumerate(((0, HNT), (HNT, NT))):
                sl = slice(a * P, b_ * P)
                nc.vector.tensor_mul(out=opre_all[:, sl], in0=opre_all[:, sl],
                                     in1=disb_sb[:, sl])
                nc.vector.tensor_reduce(out=s1p[:, ci:ci + 1],
                                        in_=opre_all[:, sl],
                                        axis=mybir.AxisListType.X,
                                        op=mybir.AluOpType.add)
                nc.scalar.activation(out=sq_all[:, sl], in_=opre_all[:, sl],
                                     func=mybir.ActivationFunctionType.Square)
                nc.vector.tensor_reduce(out=s2p[:, ci:ci + 1],
                                        in_=sq_all[:, sl],
                                        axis=mybir.AxisListType.X,
                                        op=mybir.AluOpType.add)
            nc.vector.tensor_reduce(out=st_sb[:, 0:1], in_=s1p[:],
                                    axis=mybir.AxisListType.X,
                                    op=mybir.AluOpType.add)
            nc.vector.tensor_reduce(out=st_sb[:, 1:2], in_=s2p[:],
                                    axis=mybir.AxisListType.X,
                                    op=mybir.AluOpType.add)

            # ---- BN stats allreduce + affine coefficients
            st_in = dram.tile([P, 2], dt.float32)
            st_out = dram.tile([P, 2], dt.float32, addr_space="Shared")
            st2_sb = stp.tile([P, 2], dt.float32)
            nc.sync.dma_start(out=st_in[:], in_=st_sb[:])
            nc.gpsimd.collective_compute(
                "AllReduce", mybir.AluOpType.add,
                replica_groups=[list(range(N_CORES))],
                ins=[st_in[:]], outs=[st_out[:]])
            nc.sync.dma_start(out=st2_sb[:], in_=st_out[:])

            mean = stp.tile([P, 1], dt.float32)
            nc.vector.tensor_scalar_mul(mean[:], st2_sb[:, 0:1], INV_N)
            var = stp.tile([P, 1], dt.float32)
            nc.vector.tensor_scalar_mul(var[:], st2_sb[:, 1:2], INV_N)
            nmm = stp.tile([P, 1], dt.float32)
            nc.vector.scalar_tensor_tensor(out=nmm[:], in0=mean[:], scalar=-1.0,
                                           in1=mean[:], op0=mybir.AluOpType.mult,
                                           op1=mybir.AluOpType.mult)
            nc.vector.tensor_add(out=var[:], in0=var[:], in1=nmm[:])
            nc.vector.tensor_scalar_add(var[:], var[:], BN_EPS)
            std = stp.tile([P, 1], dt.float32)
            nc.scalar.activation(out=std[:], in_=var[:],
                                 func=mybir.ActivationFunctionType.Sqrt)
            rstd = stp.tile([P, 1], dt.float32)
            nc.vector.reciprocal(out=rstd[:], in_=std[:])
            A = stp.tile([P, 1], dt.float32)
            nc.vector.tensor_mul(out=A[:], in0=gamma_sb[:], in1=rstd[:])
            B = stp.tile([P, 1], dt.float32)
            nc.vector.tensor_mul(out=B[:], in0=A[:], in1=mean[:])
            nc.vector.scalar_tensor_tensor(out=B[:], in0=B[:], scalar=-1.0,
                                           in1=beta_sb[:], op0=mybir.AluOpType.mult,
                                           op1=mybir.AluOpType.add)

            # ---- finalize: relu(A*x + B) in one in-place activation, one DMA
            nc.scalar.activation(out=opre_all[:], in_=opre_all[:],
                                 func=mybir.ActivationFunctionType.Relu,
                                 bias=B[:], scale=A[:])
            nc.sync.dma_start(out=t_out[:], in_=opre_all[:])

    nc.compile()
    return nc


# ---------------------------------------------------------------- entrypoint
def kernel(x, edge_index, W, b, gamma, beta):
    x = np.asarray(x, dtype=np.float32)
    edge_index = np.asarray(edge_index)
    W = np.asarray(W, dtype=np.float32)
    gamma = np.asarray(gamma, dtype=np.float32)
    beta = np.asarray(beta, dtype=np.float32)
    # bias cancels exactly under BatchNorm (constant per-feature shift); unused.

    plan, per_core = _plan_and_pack(x, edge_index, W, gamma, beta)
    nc = _build(plan)
    res = run_bass_kernel_spmd(nc, per_core, list(range(N_CORES)))

    N, shard = plan["N"], plan["shard"]
    out = np.empty((N, P), np.float32)
    for c in range(N_CORES):
        lo = c * shard
        hi = min((c + 1) * shard, N)
        out[lo:hi] = res.results[c]["out_t"][:, : hi - lo].T
    return out


if __name__ == "__main__":
    rng = np.random.default_rng(0)
    N, E = 2048, 8192
    x = rng.standard_normal((N, 256), dtype=np.float32)
    ei = rng.integers(0, N, (2, E)).astype(np.int64)
    W = (rng.standard_normal((256, 128), dtype=np.float32) / 16)
    g = rng.standard_normal(128).astype(np.float32) + 1.2
    be = rng.standard_normal(128).astype(np.float32)
    got = kernel(x=x, edge_index=ei, W=W, b=np.zeros(128, np.float32), gamma=g, beta=be)

    h = x @ W
    loops = np.arange(N)
    r2 = np.concatenate([ei[0], loops]); c2 = np.concatenate([ei[1], loops])
    deg = np.bincount(c2, minlength=N).astype(np.float32)
    dis = 1.0 / np.sqrt(deg)
    out = np.zeros((N, 128), np.float32)
    np.add.at(out, c2, h[r2] * (dis[r2] * dis[c2])[:, None])
    mean = out.mean(0); var = ((out - mean) ** 2).mean(0)
    ref = np.maximum(g * (out - mean) / np.sqrt(var + BN_EPS) + be, 0)
    err = np.abs(got - ref)
    print("absmax:", err.max(), "scale:", np.abs(ref).max(),
          "rel:", err.max() / np.abs(ref).max())


# revision 12
# speedup vs baseline: 1.0982x; 1.0982x over previous
"""GCN feature extractor (GCNConv + BatchNorm1d + ReLU) as a Trainium2 Bass kernel.

Distribution (8 NeuronCores):
  - Nodes are sharded row-wise across the 8 cores (graph/data parallel).
  - Each core computes m = deg^-1/2 * (x @ W) for its own node shard (PE
    matmul, bf16).  The shard is split into two halves (A: 25 tiles, B: 24
    tiles) which are AllGather'd separately so that gathers/aggregation of
    the A window overlap the B AllGather (collectives run on the CC cores,
    leaving all compute engines free).
  - Gather-table blocks are stored p-major ([part, tile, feat]) so the
    SBUF->DRAM repack is a single full-rate DMA; the host index formula
    absorbs the layout.
  - Each core owns the edges whose TARGET falls in its shard.  Per 128-target
    tile it bulk-gathers the source messages with the GpSimd descriptor-
    generated gather DMA (dma_gather), builds one-hot target-selection
    matrices on the vector engine, and reduces on the tensor engine:
        agg[f, t] += G[edge, f]^T @ onehot[edge, t]   (PSUM fp32 accumulate)
    Descriptor generation is round-robined over 4 SWDGE queues (4 Q7 core
    pairs generate concurrently).  Aggregation is two-pass (pass 1: self-loop
    + window A into SBUF; pass 2: + window B) so pass 1 runs during the B
    AllGather.
  - Self loops are folded in locally (m_local tile x identity matmul); the
    symmetric normalization factorizes as deg^-1/2[src] (folded into m) and
    deg^-1/2[tgt] (applied at the end).  +bias cancels under BatchNorm.
  - BatchNorm statistics are computed per-feature batched at the end,
    AllReduce'd across cores, applied fused with ReLU in one scalar-engine
    activation; output is written feature-major and transposed on the host.
"""

import sys

sys.path.insert(0, "/opt/trn_rl_repo")

import numpy as np
import ml_dtypes

import os
import concourse.bass as bass
import concourse.tile as tile
from concourse import bacc, mybir, library_config
from concourse.bass_utils import run_bass_kernel_spmd

N_CORES = 8
P = 128
GK = 8   # gather-DMA granularity: blocks (of 128 edges) per dma_gather call
NQ = 4   # SWDGE queues (Q7 core pairs) for gather descriptor generation
BN_EPS = 1e-5
dt = mybir.dt


# ---------------------------------------------------------------- host prep
def _plan_and_pack(x, edge_index, W, gamma, beta):
    N, IN = x.shape
    HID = W.shape[1]
    assert HID == P and IN % P == 0
    shard = (N + N_CORES - 1) // N_CORES          # nodes per core (last may be short)
    PS = ((shard + P - 1) // P) * P               # padded shard rows
    NT = PS // P                                  # 128-target tiles per core
    NTA = (NT + 1) // 2                           # tiles in window A
    NTB = NT - NTA
    RA = NTA * P                                  # rows in window A per shard
    sizA, sizB = N_CORES * RA, N_CORES * (PS - RA)
    assert sizA < 2 ** 15 and sizB < 2 ** 15, "int16 gather index overflow"

    row = np.asarray(edge_index[0], dtype=np.int64)
    col = np.asarray(edge_index[1], dtype=np.int64)

    deg = np.bincount(col, minlength=N).astype(np.float64) + 1.0
    dis = (1.0 / np.sqrt(deg)).astype(np.float32)

    # window + p-major gather-table coordinate of each node
    # table block c is [128 part, T tiles, 128 feat]; node (c, r) with
    # r = t*128 + p sits at flat row c*T*128 + p*T + t of the window table.
    cc = np.arange(N) // shard
    rr = np.arange(N) % shard
    win = (rr >= RA).astype(np.int64)
    rA = rr
    tblA = cc * RA + (rA % P) * NTA + rA // P
    rB = rr - RA
    tblB = cc * (PS - RA) + (rB % P) * NTB + rB // P
    tbl = np.where(win == 0, tblA, tblB).astype(np.int32)

    # self loops are folded locally on-device; only real edges get gathered
    order = np.argsort(col, kind="stable")
    allr = row[order]
    allc = col[order]
    src_tbl_all = tbl[allr]
    win_of = win[allr]

    tile_of = allc // shard * NT + (allc % shard) // P
    tloc_of = (allc % shard) % P

    n_tiles_total = N_CORES * NT
    key = tile_of * 2 + win_of
    sort2 = np.argsort(key, kind="stable")
    src_sorted = src_tbl_all[sort2]
    tloc_sorted = tloc_of[sort2].astype(np.int32)
    key_sorted = key[sort2]
    bounds = np.searchsorted(key_sorted, np.arange(n_tiles_total * 2 + 1))

    # shared (max over cores) block counts per (tile, window)
    nb = np.zeros((N_CORES, NT, 2), np.int64)
    for c in range(N_CORES):
        for t in range(NT):
            for w in range(2):
                k = (c * NT + t) * 2 + w
                cnt = bounds[k + 1] - bounds[k]
                nb[c, t, w] = (cnt + P - 1) // P
    nbmax = nb.max(axis=0)                         # [NT, 2]
    # stream block offsets: stream w blocks of tile t start at soff[t, w]
    soff = np.zeros((NT, 2), np.int64)
    tb = [0, 0]
    for t in range(NT):
        for w in range(2):
            soff[t, w] = tb[w]
            tb[w] += nbmax[t, w]
    TB0, TB1 = int(tb[0]), int(tb[1])
    ncalls = [(TB0 + GK - 1) // GK, (TB1 + GK - 1) // GK]
    TBpad = [ncalls[0] * GK, ncalls[1] * GK]

    per_core = []
    for c in range(N_CORES):
        streams = [np.zeros(TBpad[w] * P, np.int32) for w in range(2)]
        ctgtl = -np.ones((P, TB0 + TB1), np.float32)
        for t in range(NT):
            co = int(soff[t, 0] + soff[t, 1])
            done = 0
            for w in range(2):
                k = (c * NT + t) * 2 + w
                lo, hi = bounds[k], bounds[k + 1]
                b0 = soff[t, w]
                streams[w][b0 * P: b0 * P + (hi - lo)] = src_sorted[lo:hi]
                n = hi - lo
                if n:
                    jj = np.arange(n)
                    ctgtl[jj % P, co + done + jj // P] = tloc_sorted[lo:hi]
                done += int(nbmax[t, w])
        # pack gather indices: per call [128, GK*128/16] int16, idx j -> [16c + j%16, j//16]
        idxs = []
        for w in range(2):
            s16 = streams[w].astype(np.int16)
            a = s16.reshape(ncalls[w], GK * P // 16, 16).transpose(0, 2, 1)  # [calls, 16, cols]
            a = np.tile(a, (1, 8, 1))                                        # replicate to 128 partitions
            idxs.append(np.ascontiguousarray(a.transpose(1, 0, 2).reshape(P, -1)))

        lo_n = c * shard
        hi_n = min((c + 1) * shard, N)
        ns = hi_n - lo_n
        xs = np.zeros((IN, PS), np.float32)
        xs[:, :ns] = x[lo_n:hi_n].T
        dis_s = np.zeros(PS, np.float32)
        dis_s[:ns] = dis[lo_n:hi_n]
        per_core.append({
            "xT": xs.astype(ml_dtypes.bfloat16),
            "disb": np.ascontiguousarray(np.tile(dis_s[None, :], (P, 1))),
            "disk": np.ascontiguousarray(dis_s.reshape(NT, P).T),   # [128, NT]
            "idx0": idxs[0], "idx1": idxs[1],
            "ctgtl": ctgtl.astype(ml_dtypes.bfloat16),
            "W": np.ascontiguousarray(W.astype(ml_dtypes.bfloat16)),
            "iota": np.ascontiguousarray(
                np.tile(np.arange(P, dtype=np.float32), (P, 1)).astype(ml_dtypes.bfloat16)),
            "eye": np.eye(P, dtype=ml_dtypes.bfloat16),
            "gamma": np.ascontiguousarray(gamma.astype(np.float32).reshape(P, 1)),
            "beta": np.ascontiguousarray(beta.astype(np.float32).reshape(P, 1)),
        })

    plan = {
        "N": N, "IN": IN, "PS": PS, "NT": NT, "NTA": NTA, "shard": shard,
        "sizA": sizA, "sizB": sizB,
        "nbmax": nbmax, "soff": soff, "TB": [TB0, TB1], "ncalls": ncalls,
        "KC": IN // P,
    }
    return plan, per_core


# ---------------------------------------------------------------- bass build
def _build(plan):
    N, IN, PS, NT = plan["N"], plan["IN"], plan["PS"], plan["NT"]
    KC = plan["KC"]
    NTA = plan["NTA"]
    NTB = NT - NTA
    sizA, sizB = plan["sizA"], plan["sizB"]
    nbmax, soff = plan["nbmax"], plan["soff"]
    ncalls = plan["ncalls"]
    NIDX = GK * P

    nc = bacc.Bacc("TRN2", target_bir_lowering=False, debug=False,
                   num_devices=N_CORES, num_swdge_queues=NQ)
    t_xT = nc.dram_tensor("xT", [IN, PS], dt.bfloat16, kind="ExternalInput").ap()
    t_W = nc.dram_tensor("W", [IN, P], dt.bfloat16, kind="ExternalInput").ap()
    t_disb = nc.dram_tensor("disb", [P, PS], dt.float32, kind="ExternalInput").ap()
    t_disk = nc.dram_tensor("disk", [P, NT], dt.float32, kind="ExternalInput").ap()
    t_idx = [nc.dram_tensor(f"idx{w}", [P, ncalls[w] * NIDX // 16], dt.int16,
                            kind="ExternalInput").ap() for w in range(2)]
    t_ctgtl = nc.dram_tensor("ctgtl", [P, plan["TB"][0] + plan["TB"][1]],
                             dt.bfloat16, kind="ExternalInput").ap()
    t_iota = nc.dram_tensor("iota", [P, P], dt.bfloat16, kind="ExternalInput").ap()
    t_eye = nc.dram_tensor("eye", [P, P], dt.bfloat16, kind="ExternalInput").ap()
    t_gamma = nc.dram_tensor("gamma", [P, 1], dt.float32, kind="ExternalInput").ap()
    t_beta = nc.dram_tensor("beta", [P, 1], dt.float32, kind="ExternalInput").ap()
    t_out = nc.dram_tensor("out_t", [P, PS], dt.float32, kind="ExternalOutput").ap()

    INV_N = 1.0 / N

    with tile.TileContext(nc) as tc:
        nc.gpsimd.load_library(library_config.mlp)
        with tc.tile_pool(name="consts", bufs=1) as cst, \
             tc.tile_pool(name="gp0", bufs=12) as gp0, \
             tc.tile_pool(name="gp1", bufs=12) as gp1, \
             tc.tile_pool(name="ohp", bufs=3) as ohp, \
             tc.tile_pool(name="big", bufs=1) as big, \
             tc.tile_pool(name="ep", bufs=3) as ep, \
             tc.tile_pool(name="hps", bufs=1, space="PSUM") as hps, \
             tc.tile_pool(name="aps", bufs=6, space="PSUM") as aps, \
             tc.tile_pool(name="stp", bufs=1) as stp, \
             tc.tile_pool(name="dram", bufs=1, space="DRAM") as dram:

            # ---- constants to SBUF
            W_sb = cst.tile([P, KC, P], dt.bfloat16)
            for k in range(KC):
                nc.sync.dma_start(out=W_sb[:, k, :], in_=t_W[k * P:(k + 1) * P, :])
            disk_sb = cst.tile([P, NT], dt.float32)
            nc.sync.dma_start(out=disk_sb[:], in_=t_disk[:])
            # bulk xT load: 2 big DMAs instead of 98 tile loads
            # (allocated from the rotating "big" pool so the square buffer
            #  can reuse the same bytes once phase B has consumed xT)
            xT_sb = big.tile([P, KC, PS], dt.bfloat16, name="big")
            for q in range(KC):
                nc.sync.dma_start(out=xT_sb[:, q, :],
                                  in_=t_xT[q * P:(q + 1) * P, :])

            # ---- phase B: m = dis * (x @ W), bf16, kept in SBUF (p-major)
            m_all = cst.tile([P, NT, P], dt.bfloat16)

            def phase_b(t0, t1):
                for k in range(t0, t1):
                    h_ps = hps.tile([P, P], dt.float32, name="hps")
                    for q in range(KC):
                        nc.tensor.matmul(out=h_ps[:],
                                         lhsT=xT_sb[:, q, k * P:(k + 1) * P],
                                         rhs=W_sb[:, q, :],
                                         start=(q == 0), stop=(q == KC - 1))
                    nc.vector.tensor_scalar(out=m_all[:, k, :], in0=h_ps[:],
                                            scalar1=disk_sb[:, k:k + 1],
                                            scalar2=None,
                                            op0=mybir.AluOpType.mult)

            # window A: compute, repack (one DMA, p-major), AllGather
            m_ccA = dram.tile([P, NTA * P], dt.bfloat16)
            m_fullA = dram.tile([sizA, P], dt.bfloat16, addr_space="Shared")
            phase_b(0, NTA)
            nc.sync.dma_start(out=m_ccA[:], in_=m_all[:, 0:NTA, :])
            nc.gpsimd.collective_compute(
                "AllGather", mybir.AluOpType.bypass,
                replica_groups=[list(range(N_CORES))],
                ins=[m_ccA[:]], outs=[m_fullA[:]])

            # gather/agg metadata loads issue only now, so they don't delay
            # the phase-B inputs and the first AllGather on the DMA queues
            iota_sb = cst.tile([P, P], dt.bfloat16)
            nc.sync.dma_start(out=iota_sb[:], in_=t_iota[:])
            eye_sb = cst.tile([P, P], dt.bfloat16)
            nc.sync.dma_start(out=eye_sb[:], in_=t_eye[:])
            idx_sb = [cst.tile([P, ncalls[w] * NIDX // 16], dt.int16, name=f"idx{w}")
                      for w in range(2)]
            for w in range(2):
                nc.sync.dma_start(out=idx_sb[w][:], in_=t_idx[w][:])
            TBC = plan["TB"][0] + plan["TB"][1]
            ctgtl_sb = cst.tile([P, TBC], dt.bfloat16)
            nc.sync.dma_start(out=ctgtl_sb[:], in_=t_ctgtl[:])
            disb_sb = cst.tile([P, PS], dt.float32)
            nc.sync.dma_start(out=disb_sb[:], in_=t_disb[:])
            gamma_sb = cst.tile([P, 1], dt.float32)
            nc.sync.dma_start(out=gamma_sb[:], in_=t_gamma[:])
            beta_sb = cst.tile([P, 1], dt.float32)
            nc.sync.dma_start(out=beta_sb[:], in_=t_beta[:])

            # window B
            m_ccB = dram.tile([P, NTB * P], dt.bfloat16)
            m_fullB = dram.tile([sizB, P], dt.bfloat16, addr_space="Shared")
            phase_b(NTA, NT)
            nc.sync.dma_start(out=m_ccB[:], in_=m_all[:, NTA:NT, :])
            nc.gpsimd.collective_compute(
                "AllGather", mybir.AluOpType.bypass,
                replica_groups=[list(range(N_CORES))],
                ins=[m_ccB[:]], outs=[m_fullB[:]])

            # ---- gather pipelines (two windows, round-robin SWDGE queues)
            g_tiles = [[], []]
            gpools = [gp0, gp1]
            m_wins = [m_fullA, m_fullB]
            qctr = 0

            def issue_gathers(w):
                nonlocal qctr
                for cidx in range(ncalls[w]):
                    gt = gpools[w].tile([P, GK, P], dt.bfloat16, name=f"g{w}")
                    nc.gpsimd.dma_gather(
                        out_ap=gt[:],
                        in_ap=m_wins[w][:],
                        idxs_ap=idx_sb[w][:, cidx * NIDX // 16:(cidx + 1) * NIDX // 16],
                        num_idxs=NIDX, num_idxs_reg=NIDX, elem_size=P,
                        queue_num=qctr % NQ)
                    qctr += 1
                    g_tiles[w].append(gt)

            opre_all = stp.tile([P, NT * P], dt.float32)
            OHMX = int(nbmax.sum(axis=1).max())

            issue_gathers(0)
            issue_gathers(1)

            for t in range(NT):
                nb0, nb1 = int(nbmax[t, 0]), int(nbmax[t, 1])
                nbt = nb0 + nb1
                co = int(soff[t, 0] + soff[t, 1])
                ps_t = aps.tile([P, P], dt.float32, name="agg")
                # self-loop fold: out[f, t] += m_local[t, f]
                nc.tensor.matmul(out=ps_t[:], lhsT=m_all[:, t, :],
                                 rhs=eye_sb[:], start=True, stop=(nbt == 0))
                if nbt:
                    oh = ohp.tile([P, OHMX, P], dt.bfloat16, name="oh")
                    nc.vector.tensor_tensor(
                        out=oh[:, 0:nbt, :],
                        in0=ctgtl_sb[:, co:co + nbt].unsqueeze(2)
                            .to_broadcast([P, nbt, P]),
                        in1=iota_sb[:].unsqueeze(1).to_broadcast([P, nbt, P]),
                        op=mybir.AluOpType.is_equal)
                    for b in range(nbt):
                        w = 0 if b < nb0 else 1
                        j = int(soff[t, w]) + (b if w == 0 else b - nb0)
                        gt = g_tiles[w][j // GK]
                        nc.tensor.matmul(
                            out=ps_t[:], lhsT=gt[:, j % GK, :],
                            rhs=oh[:, b, :],
                            start=False, stop=(b == nbt - 1))
                nc.vector.tensor_copy(out=opre_all[:, t * P:(t + 1) * P],
                                      in_=ps_t[:])

            # ---- scale by dis[tgt], batched stats
            nc.vector.tensor_mul(out=opre_all[:], in0=opre_all[:],
                                 in1=disb_sb[:])
            st_sb = stp.tile([P, 2], dt.float32)
            nc.vector.tensor_reduce(out=st_sb[:, 0:1], in_=opre_all[:],
                                    axis=mybir.AxisListType.X,
                                    op=mybir.AluOpType.add)
            sq_all = big.tile([P, NT * P], dt.float32, name="big")
            nc.scalar.activation(out=sq_all[:], in_=opre_all[:],
                                 func=mybir.ActivationFunctionType.Square)
            nc.vector.tensor_reduce(out=st_sb[:, 1:2], in_=sq_all[:],
                                    axis=mybir.AxisListType.X,
                                    op=mybir.AluOpType.add)

            # ---- BN stats allreduce + affine coefficients
            st_in = dram.tile([P, 2], dt.float32)
            st_out = dram.tile([P, 2], dt.float32, addr_space="Shared")
            st2_sb = stp.tile([P, 2], dt.float32)
            nc.sync.dma_start(out=st_in[:], in_=st_sb[:])
            nc.gpsimd.collective_compute(
                "AllReduce", mybir.AluOpType.add,
                replica_groups=[list(range(N_CORES))],
                ins=[st_in[:]], outs=[st_out[:]])
            nc.sync.dma_start(out=st2_sb[:], in_=st_out[:])

            mean = stp.tile([P, 1], dt.float32)
            nc.vector.tensor_scalar_mul(mean[:], st2_sb[:, 0:1], INV_N)
            var = stp.tile([P, 1], dt.float32)
            nc.vector.tensor_scalar_mul(var[:], st2_sb[:, 1:2], INV_N)
            nmm = stp.tile([P, 1], dt.float32)
            nc.vector.scalar_tensor_tensor(out=nmm[:], in0=mean[:], scalar=-1.0,
                                           in1=mean[:], op0=mybir.AluOpType.mult,
                                           op1=mybir.AluOpType.mult)
            nc.vector.tensor_add(out=var[:], in0=var[:], in1=nmm[:])
            nc.vector.tensor_scalar_add(var[:], var[:], BN_EPS)
            std = stp.tile([P, 1], dt.float32)
            nc.scalar.activation(out=std[:], in_=var[:],
                                 func=mybir.ActivationFunctionType.Sqrt)
            rstd = stp.tile([P, 1], dt.float32)
            nc.vector.reciprocal(out=rstd[:], in_=std[:])
            A = stp.tile([P, 1], dt.float32)
            nc.vector.tensor_mul(out=A[:], in0=gamma_sb[:], in1=rstd[:])
            B = stp.tile([P, 1], dt.float32)
            nc.vector.tensor_mul(out=B[:], in0=A[:], in1=mean[:])
            nc.vector.scalar_tensor_tensor(out=B[:], in0=B[:], scalar=-1.0,
                                           in1=beta_sb[:], op0=mybir.AluOpType.mult,
                                           op1=mybir.AluOpType.add)

            # ---- finalize: relu(A*x + B) in one in-place activation, one DMA
            nc.scalar.activation(out=opre_all[:], in_=opre_all[:],
                                 func=mybir.ActivationFunctionType.Relu,
                                 bias=B[:], scale=A[:])
            nc.sync.dma_start(out=t_out[:], in_=opre_all[:])

    nc.compile()
    return nc


# ---------------------------------------------------------------- entrypoint
def kernel(x, edge_index, W, b, gamma, beta):
    x = np.asarray(x, dtype=np.float32)
    edge_index = np.asarray(edge_index)
    W = np.asarray(W, dtype=np.float32)
    gamma = np.asarray(gamma, dtype=np.float32)
    beta = np.asarray(beta, dtype=np.float32)
    # bias cancels exactly under BatchNorm (constant per-feature shift); unused.

    plan, per_core = _plan_and_pack(x, edge_index, W, gamma, beta)
    nc = _build(plan)
    res = run_bass_kernel_spmd(nc, per_core, list(range(N_CORES)))

    N, shard = plan["N"], plan["shard"]
    out = np.empty((N, P), np.float32)
    for c in range(N_CORES):
        lo = c * shard
        hi = min((c + 1) * shard, N)
        out[lo:hi] = res.results[c]["out_t"][:, : hi - lo].T
    return out


if __name__ == "__main__":
    rng = np.random.default_rng(0)
    N, E = 2048, 8192
    x = rng.standard_normal((N, 256), dtype=np.float32)
    ei = rng.integers(0, N, (2, E)).astype(np.int64)
    W = (rng.standard_normal((256, 128), dtype=np.float32) / 16)
    g = rng.standard_normal(128).astype(np.float32) + 1.2
    be = rng.standard_normal(128).astype(np.float32)
    got = kernel(x=x, edge_index=ei, W=W, b=np.zeros(128, np.float32), gamma=g, beta=be)

    h = x @ W
    loops = np.arange(N)
    r2 = np.concatenate([ei[0], loops]); c2 = np.concatenate([ei[1], loops])
    deg = np.bincount(c2, minlength=N).astype(np.float32)
    dis = 1.0 / np.sqrt(deg)
    out = np.zeros((N, 128), np.float32)
    np.add.at(out, c2, h[r2] * (dis[r2] * dis[c2])[:, None])
    mean = out.mean(0); var = ((out - mean) ** 2).mean(0)
    ref = np.maximum(g * (out - mean) / np.sqrt(var + BN_EPS) + be, 0)
    err = np.abs(got - ref)
    print("absmax:", err.max(), "scale:", np.abs(ref).max(),
          "rel:", err.max() / np.abs(ref).max())
